# revision 12
# baseline (speedup 1.0000x reference)
"""Causal single-head attention on 8 Trainium2 NeuronCores.

Problem: embedding_word [4, 2048, 1024] fp32; w_q/w_k/w_v [1024, 1024] fp32.
  q = x @ w_q; k = x @ w_k; v = x @ w_v
  out = softmax(causal_mask(q k^T) / 32) @ v          per batch.

Sharding: 4 batches x 2 key-shards = 8 cores (SPMD, one program).
Core (b, p) handles batch b and the interleaved key blocks
{128*(2i+p) : i in 0..7} (1024 keys), for ALL 2048 query rows, producing
the *unnormalized* attention output u = sum_s exp(score) * v[s] and the
per-row sum of exp.  Host combines the two key-shards per batch:
  out = (u_p0 + u_p1) / (s_p0 + s_p1).
Scores are bounded (|score/32| < ~2), so softmax without max-subtraction
is numerically safe and the partial sums combine linearly.

Two tricks vs a plain bf16 implementation:

1. M-folding: scores = q k^T = x (w_q w_k^T) x^T.  The host precomputes
   M = w_q w_k^T in fp64; the kernel computes h = x_keys @ M^T (one
   k-projection-sized matmul) and scores = x @ h^T.  The entire
   q-projection (a third of all projection FLOPs, and the only
   projection needing all 2048 tokens) disappears.

2. fp8-e4m3 DoubleRow scores: the score matmuls (a quarter of the
   attention FLOPs) run with 2 contraction elements per PE cell-cycle
   (K=256 per instruction) on x8 = fp8(16x) and h8 = fp8(16h).  Score
   error ~2.5% of logit scale shifts softmax weights by ~2.5% relative,
   which the consistent e/sums normalization mostly cancels; measured
   end-to-end error is ~1e-2 absmax-relative / ~1.2e-2 Frobenius.
   Everything touching v stays bf16 (v-quantization error does NOT
   average out of a relative-error metric).

Layout: the score moving operand x8 is in ORIGINAL token order, so row
tiles (TJ=512 queries) are contiguous slices, output rows land in
original order (no host unpermute), and causal masks are simple shifted
triangles baked per-core by the host ([128, 2*512]: diagonal slot
2J+d is masked with m16[:, 512d : 512d+512], = (q >= r + 256d + 128p)).
The e/sums/AV path is uniform bf16 for all row tiles; AV matmuls that
are fully masked out ((d=1 diagonal slot) x (query c-blocks 0,1)) are
skipped on both parities.
"""

import numpy as np
import ml_dtypes

try:
    import concourse.bass as bass  # noqa: F401
except ImportError:  # pragma: no cover
    import sys

    sys.path.insert(0, "/opt/trn_rl_repo")
    import concourse.bass as bass  # noqa: F401

from contextlib import ExitStack

import concourse.tile as tile
from concourse import bacc, mybir
from concourse.bass_utils import run_bass_kernel_spmd

B = 4
T = 2048
D = 1024
P = 128
KT = D // P  # 8 contraction subtiles of 128
NSLOT = 8  # key slots per core (each 128 keys)
TJ = 512  # query rows per attention tile
NJ = T // TJ  # 4 row tiles
BF16 = mybir.dt.bfloat16
F8 = mybir.dt.float8e4
F32 = mybir.dt.float32
DR = mybir.MatmulPerfMode.DoubleRow

SX = 16.0  # x8 = SX * x^T
SH = 16.0  # h8 = SH * h
EXP_SCALE = (1.0 / 32.0) / (SX * SH)  # score psum -> logits

_NC_CACHE = {}


def _key_blocks(p):
    """Key slot i (0..7) -> original 128-row block index."""
    return [2 * i + p for i in range(NSLOT)]


def _build_program():
    nc = bacc.Bacc(
        "TRN2",
        target_bir_lowering=False,
        debug=False,
        enable_asserts=False,
        num_devices=8,
    )
    x8q = nc.dram_tensor("x8q", [D, T], F8, kind="ExternalInput").ap()
    xt16k = nc.dram_tensor("xt16k", [D, NSLOT * P], BF16, kind="ExternalInput").ap()
    mt16 = nc.dram_tensor("mt16", [D, D], BF16, kind="ExternalInput").ap()
    wv16 = nc.dram_tensor("wv16", [D, D], BF16, kind="ExternalInput").ap()
    m16 = nc.dram_tensor("m16", [P, 2 * TJ], BF16, kind="ExternalInput").ap()
    out_u = nc.dram_tensor("out_u", [T, D], BF16, kind="ExternalOutput").ap()
    sums = nc.dram_tensor("sums", [1, T], F32, kind="ExternalOutput").ap()

    with tile.TileContext(nc) as tc, ExitStack() as ctx:
        _emit(ctx, tc, x8q, xt16k, mt16, wv16, m16, out_u, sums)
    nc.compile()
    return nc


def _emit(ctx, tc, x8q, xt16k, mt16, wv16, m16, out_u, sums):
    nc = tc.nc

    const = ctx.enter_context(tc.tile_pool(name="const", bufs=1))
    big = ctx.enter_context(tc.tile_pool(name="big", bufs=1))
    epool = ctx.enter_context(tc.tile_pool(name="epool", bufs=15))
    outp = ctx.enter_context(tc.tile_pool(name="outp", bufs=6))
    ps_w = ctx.enter_context(tc.tile_pool(name="ps_w", bufs=2, space="PSUM"))
    ps_av = ctx.enter_context(tc.tile_pool(name="ps_av", bufs=5, space="PSUM"))
    ps_s = ctx.enter_context(tc.tile_pool(name="ps_s", bufs=1, space="PSUM"))

    # Persistent SBUF tensors (layout [128 partitions, outer, free]).
    x8q_sb = big.tile([P, KT, T], F8)  # 16*x^T [dm_p, dm_o, t(ORIG)]
    xt16k_sb = big.tile([P, KT, NSLOT * P], BF16)  # x^T keys [dm_p, dm_o, s]
    mt16_sb = big.tile([P, KT, D], BF16)  # M^T    [di_p, di_o, do]
    wv16_sb = big.tile([P, KT, D], BF16)
    h8_sb = big.tile([P, KT, NSLOT * P], F8)  # 16*h^T [do_p, do_o, s]
    v16_sb = big.tile([P, NSLOT, D], BF16)  # v      [s_p, s_o, dv]
    m16_sb = const.tile([P, 2 * TJ], BF16)
    ones16 = const.tile([P, 1], BF16)
    s_all = const.tile([1, NJ, TJ], F32)
    # out_u viewed [J, p, c, n]: row J*512 + c*128 + p.
    out_r = out_u.rearrange("(j c p) n -> j p c n", c=4, p=P)

    nc.vector.memset(ones16[:], 1.0)
    # Warm-up: keep the PE busy while the first input DMAs land so the
    # HAM clock gate reaches 2.4 GHz before real work starts.
    warm_sb = const.tile([P, 512], BF16)
    nc.vector.memset(warm_sb[:], 0.0)
    # ~28 x 427ns(cold) covers DMA-start latency (up to ~12us observed) so
    # the PE never idles >3.4us early on -- an early idle window would drop
    # the HAM clock gate back to 1.2 GHz and can poison the whole run.
    warm_ps = ps_w.tile([P, 512], F32, tag="ps_work", name="warm")
    for _ in range(28):
        nc.tensor.matmul(warm_ps[:1, :], ones16[:], warm_sb[:], start=True, stop=True)

    # Input DMA, ordered by first use; two HWDGE rings (sync/scalar).
    mt16_r = mt16.rearrange("(o p) n -> p o n", p=P)
    xt16k_r = xt16k.rearrange("(o p) n -> p o n", p=P)
    x8q_r = x8q.rearrange("(o p) n -> p o n", p=P)
    wv16_r = wv16.rearrange("(o p) n -> p o n", p=P)
    nc.sync.dma_start(mt16_sb[:, :, :128], mt16_r[:, :, :128])
    nc.scalar.dma_start(xt16k_sb[:, :, :512], xt16k_r[:, :, :512])
    nc.sync.dma_start(mt16_sb[:, :, 128:512], mt16_r[:, :, 128:512])
    nc.sync.dma_start(mt16_sb[:, :, 512:], mt16_r[:, :, 512:])
    nc.scalar.dma_start(xt16k_sb[:, :, 512:], xt16k_r[:, :, 512:])
    nc.scalar.dma_start(x8q_sb[:, :, :1024], x8q_r[:, :, :1024])
    nc.sync.dma_start(m16_sb[:], m16[:])
    nc.sync.dma_start(wv16_sb[:, :, :512], wv16_r[:, :, :512])
    nc.scalar.dma_start(x8q_sb[:, :, 1024:], x8q_r[:, :, 1024:])
    nc.sync.dma_start(wv16_sb[:, :, 512:], wv16_r[:, :, 512:])

    def proj16(lhs_sb, rhs_sb, m, n, name):
        """bf16 psum[m-block(128), n-block(512)] = lhsT.T @ rhs over dm."""
        ps = ps_w.tile([P, 512], F32, tag="ps_work", name=name)
        for kt in range(KT):
            nc.tensor.matmul(
                ps[:],
                lhs_sb[:, kt, m * P : (m + 1) * P],
                rhs_sb[:, kt, n * 512 : (n + 1) * 512],
                start=(kt == 0),
                stop=(kt == KT - 1),
            )
        return ps

    # --- h projection: h8[do, s] = 16 * (M @ x_s), this core's 1024 keys.
    for n in range(2):
        for m in range(KT):
            ps = proj16(mt16_sb, xt16k_sb, m, n, f"hp_{m}_{n}")
            nc.vector.tensor_scalar_mul(
                h8_sb[:, m, n * 512 : (n + 1) * 512], ps[:], SH
            )

    # --- attention row tile J (TJ=512 queries, orig rows 512J..512J+511):
    # key slots 0..2J+1; slots {2J, 2J+1} are diagonal (host-baked masks,
    # shift 256d + 128p).
    def scores_exp(J):
        es = []
        for s in range(2 * J + 2):
            sc = ps_w.tile([P, TJ], F32, tag="ps_work", name=f"sc_{J}_{s}")
            for kp in range(KT // 2):
                nc.tensor.matmul(
                    sc[:],
                    h8_sb[:, 2 * kp : 2 * kp + 2, s * P : (s + 1) * P],
                    x8q_sb[:, 2 * kp : 2 * kp + 2, J * TJ : (J + 1) * TJ],
                    start=(kp == 0),
                    stop=(kp == KT // 2 - 1),
                    perf_mode=DR,
                )
            e16 = epool.tile([P, TJ], BF16, tag="e16", name=f"e_{J}_{s}")
            nc.scalar.activation(
                e16[:], sc[:], mybir.ActivationFunctionType.Exp, scale=EXP_SCALE
            )
            d = s - 2 * J
            if d >= 0:  # diagonal slot
                nc.gpsimd.tensor_tensor(
                    e16[:], e16[:], m16_sb[:, d * TJ : (d + 1) * TJ],
                    mybir.AluOpType.mult,
                )
            es.append(e16)
        return es

    def sums_av_drain(J, es):
        nslots = 2 * J + 2
        sums_ps = ps_s.tile([1, TJ], F32, tag="ps_sums", name=f"su_{J}")
        for s in range(nslots):
            nc.tensor.matmul(
                sums_ps[:],
                ones16[:],
                es[s][:],
                start=(s == 0),
                stop=(s == nslots - 1),
            )
        nc.vector.tensor_copy(s_all[:, J, :], sums_ps[:])
        if J == NJ - 1:
            nc.sync.dma_start(sums[:], s_all[:, :, :])

        for dvh in range(2):
            av = [
                ps_av.tile([P, 512], F32, tag="ps_av", name=f"av_{J}_{c}_{dvh}")
                for c in range(4)
            ]
            for s in range(nslots):
                last_d1 = s == nslots - 1  # d=1 diagonal: c-blocks 0,1 all-zero
                for c in range(4):
                    if last_d1 and c < 2:
                        continue
                    nc.tensor.matmul(
                        av[c][:],
                        es[s][:, c * P : (c + 1) * P],
                        v16_sb[:, s, dvh * 512 : (dvh + 1) * 512],
                        start=(s == 0),
                        stop=(s == (nslots - 2 if c < 2 else nslots - 1)),
                    )
            # Stage all four 128-row c-blocks in one [P, 4, 512] tile and
            # issue a single coalesced output DMA per (J, dvh).
            ob = outp.tile([P, 4, 512], BF16, tag="o_sb", name=f"o_{J}_{dvh}")
            for c in range(4):
                if c % 2:
                    nc.scalar.activation(
                        ob[:, c, :], av[c][:], mybir.ActivationFunctionType.Copy
                    )
                else:
                    nc.vector.tensor_copy(ob[:, c, :], av[c][:])
            dma = nc.sync if dvh else nc.scalar
            dma.dma_start(
                out_r[J, :, :, dvh * 512 : (dvh + 1) * 512], ob[:]
            )

    # Pipeline: scores(J) stay one step ahead of sums/AV(J-1) so exp
    # latency hides under PE work; v-projection slots in before the
    # first AV consumer.
    e0 = scores_exp(0)

    # --- v projection: v16[s, dv] (bf16).
    for m in range(NSLOT):
        for n in range(2):
            ps = proj16(xt16k_sb, wv16_sb, m, n, f"vp_{m}_{n}")
            nc.vector.tensor_copy(v16_sb[:, m, n * 512 : (n + 1) * 512], ps[:])

    e1 = scores_exp(1)
    sums_av_drain(0, e0)
    e2 = scores_exp(2)
    sums_av_drain(1, e1)
    e3 = scores_exp(3)
    sums_av_drain(2, e2)
    sums_av_drain(3, e3)


def _shard_inputs(x, wq, wk, wv):
    bf = ml_dtypes.bfloat16
    f8 = ml_dtypes.float8_e4m3
    M = wq.astype(np.float64) @ wk.astype(np.float64).T  # [do, di]
    mt16 = np.ascontiguousarray(M.T.astype(bf))  # [di, do]
    wv16 = np.ascontiguousarray(wv.astype(bf))
    l = np.arange(P)
    in_maps = []
    for b in range(B):
        for p in range(2):
            krows = np.concatenate(
                [np.arange(blk * P, blk * P + P) for blk in _key_blocks(p)]
            )
            xt = x[b].T  # [D, T] original order
            q = np.arange(TJ)
            m = np.concatenate(
                [
                    (q[None, :] >= l[:, None] + 256 * d + 128 * p).astype(np.float32)
                    for d in range(2)
                ],
                axis=1,
            )  # [128, 1024]
            in_maps.append(
                {
                    "x8q": np.ascontiguousarray((SX * xt).astype(f8)),
                    "xt16k": np.ascontiguousarray(xt[:, krows].astype(bf)),
                    "mt16": mt16,
                    "wv16": wv16,
                    "m16": np.ascontiguousarray(m.astype(bf)),
                }
            )
    return in_maps


def run(embedding_word, w_q, w_k, w_v, **spmd_kwargs):
    x = np.asarray(embedding_word, dtype=np.float32)
    assert x.shape == (B, T, D), x.shape
    if "nc" not in _NC_CACHE:
        _NC_CACHE["nc"] = _build_program()
    nc = _NC_CACHE["nc"]
    in_maps = _shard_inputs(
        x,
        np.asarray(w_q, np.float32),
        np.asarray(w_k, np.float32),
        np.asarray(w_v, np.float32),
    )
    # The accelerator occasionally reports a transient unrecoverable state
    # on the first touch from a fresh process; retry a couple of times.
    last_err = None
    for attempt in range(3):
        try:
            res = run_bass_kernel_spmd(
                nc, in_maps, core_ids=list(range(8)), **spmd_kwargs
            )
            break
        except Exception as err:  # pragma: no cover
            last_err = err
            import time

            time.sleep(5.0 * (attempt + 1))
    else:
        raise last_err

    out = np.empty((B, T, D), np.float32)
    for b in range(B):
        u = res.results[2 * b]["out_u"].astype(np.float32) + res.results[
            2 * b + 1
        ]["out_u"].astype(np.float32)
        s = (
            res.results[2 * b]["sums"].reshape(T)
            + res.results[2 * b + 1]["sums"].reshape(T)
        )
        out[b] = u / s[:, None]
    return out, res


def kernel(embedding_word, w_q, w_k, w_v):
    out, _ = run(embedding_word, w_q, w_k, w_v)
    return out


# revision 16
# speedup vs baseline: 1.1681x; 1.1681x over previous
"""Causal single-head attention on 8 Trainium2 NeuronCores.

Problem: embedding_word [4, 2048, 1024] fp32; w_q/w_k/w_v [1024, 1024] fp32.
  q = x @ w_q; k = x @ w_k; v = x @ w_v
  out = softmax(causal_mask(q k^T) / 32) @ v          per batch.

Sharding: 4 batches x 2 key-shards = 8 cores (SPMD, one program).
Core (b, p) handles batch b and the interleaved key blocks
{128*(2i+p) : i in 0..7} (1024 keys), for ALL 2048 query rows, producing
the *unnormalized* attention output u = sum_s exp(score) * v[s] and the
per-row sum of exp.  Host combines the two key-shards per batch:
  out = (u_p0 + u_p1) / (s_p0 + s_p1).
Scores are bounded (|score/32| < ~2), so softmax without max-subtraction
is numerically safe and the partial sums combine linearly.

Two tricks vs a plain bf16 implementation:

1. M-folding: scores = q k^T = x (w_q w_k^T) x^T.  The host precomputes
   M = w_q w_k^T in fp64; the kernel computes h = x_keys @ M^T (one
   k-projection-sized matmul) and scores = x @ h^T.  The entire
   q-projection (a third of all projection FLOPs, and the only
   projection needing all 2048 tokens) disappears.

2. fp8-e4m3 DoubleRow scores: the score matmuls (a quarter of the
   attention FLOPs) run with 2 contraction elements per PE cell-cycle
   (K=256 per instruction) on x8 = fp8(16x) and h8 = fp8(16h).  Score
   error ~2.5% of logit scale shifts softmax weights by ~2.5% relative,
   which the consistent e/sums normalization mostly cancels; measured
   end-to-end error is ~1e-2 absmax-relative / ~1.2e-2 Frobenius.
   Everything touching v stays bf16 (v-quantization error does NOT
   average out of a relative-error metric).

Layout: the score moving operand x8 is in ORIGINAL token order, so row
tiles (TJ=512 queries) are contiguous slices, output rows land in
original order (no host unpermute), and causal masks are simple shifted
triangles baked per-core by the host ([128, 2*512]: diagonal slot
2J+d is masked with m16[:, 512d : 512d+512], = (q >= r + 256d + 128p)).
The e/sums/AV path is uniform bf16 for all row tiles; AV matmuls that
are fully masked out ((d=1 diagonal slot) x (query c-blocks 0,1)) are
skipped on both parities.
"""

import numpy as np
import ml_dtypes

try:
    import concourse.bass as bass  # noqa: F401
except ImportError:  # pragma: no cover
    import sys

    sys.path.insert(0, "/opt/trn_rl_repo")
    import concourse.bass as bass  # noqa: F401

from contextlib import ExitStack

import concourse.tile as tile
from concourse import bacc, mybir
from concourse.bass_utils import run_bass_kernel_spmd

B = 4
T = 2048
D = 1024
P = 128
KT = D // P  # 8 contraction subtiles of 128
NSLOT = 8  # key slots per core (each 128 keys)
TJ = 512  # query rows per attention tile
NJ = T // TJ  # 4 row tiles
BF16 = mybir.dt.bfloat16
F8 = mybir.dt.float8e4
F32 = mybir.dt.float32
DR = mybir.MatmulPerfMode.DoubleRow

SX = 16.0  # x8 = SX * x^T
SH = 16.0  # h8 = SH * h
EXP_SCALE = (1.0 / 32.0) / (SX * SH)  # score psum -> logits

_NC_CACHE = {}


def _key_blocks(p):
    """Key slot i (0..7) -> original 128-row block index."""
    return [2 * i + p for i in range(NSLOT)]


def _build_program():
    nc = bacc.Bacc(
        "TRN2",
        target_bir_lowering=False,
        debug=False,
        enable_asserts=False,
        num_devices=8,
    )
    x8q = nc.dram_tensor("x8q", [D, T], F8, kind="ExternalInput").ap()
    xt16k = nc.dram_tensor("xt16k", [D, NSLOT * P], BF16, kind="ExternalInput").ap()
    mt16 = nc.dram_tensor("mt16", [D, D], BF16, kind="ExternalInput").ap()
    wv16 = nc.dram_tensor("wv16", [D, D], BF16, kind="ExternalInput").ap()
    m16 = nc.dram_tensor("m16", [P, 2 * TJ], BF16, kind="ExternalInput").ap()
    out_u = nc.dram_tensor("out_u", [T, D], BF16, kind="ExternalOutput").ap()
    sums = nc.dram_tensor("sums", [1, T], F32, kind="ExternalOutput").ap()

    with tile.TileContext(nc) as tc, ExitStack() as ctx:
        _emit(ctx, tc, x8q, xt16k, mt16, wv16, m16, out_u, sums)
    nc.compile()
    return nc


def _emit(ctx, tc, x8q, xt16k, mt16, wv16, m16, out_u, sums):
    nc = tc.nc

    const = ctx.enter_context(tc.tile_pool(name="const", bufs=1))
    big = ctx.enter_context(tc.tile_pool(name="big", bufs=1))
    epool = ctx.enter_context(tc.tile_pool(name="epool", bufs=15))
    outp = ctx.enter_context(tc.tile_pool(name="outp", bufs=6))
    ps_w = ctx.enter_context(tc.tile_pool(name="ps_w", bufs=2, space="PSUM"))
    ps_av = ctx.enter_context(tc.tile_pool(name="ps_av", bufs=5, space="PSUM"))
    ps_s = ctx.enter_context(tc.tile_pool(name="ps_s", bufs=1, space="PSUM"))

    # Persistent SBUF tensors (layout [128 partitions, outer, free]).
    x8q_sb = big.tile([P, KT, T], F8)  # 16*x^T [dm_p, dm_o, t(ORIG)]
    xt16k_sb = big.tile([P, KT, NSLOT * P], BF16)  # x^T keys [dm_p, dm_o, s]
    mt16_sb = big.tile([P, KT, D], BF16)  # M^T    [di_p, di_o, do]
    wv16_sb = big.tile([P, KT, D], BF16)
    h8_sb = big.tile([P, KT, NSLOT * P], F8)  # 16*h^T [do_p, do_o, s]
    v16_sb = big.tile([P, NSLOT, D], BF16)  # v      [s_p, s_o, dv]
    m16_sb = const.tile([P, 2 * TJ], BF16)
    ones16 = const.tile([P, 1], BF16)
    s_all = const.tile([1, NJ, TJ], F32)

    nc.vector.memset(ones16[:], 1.0)
    # Warm-up: keep the PE busy while the first input DMAs land so the
    # HAM clock gate reaches 2.4 GHz before real work starts.
    warm_sb = const.tile([P, 512], BF16)
    nc.vector.memset(warm_sb[:], 0.0)
    # ~28 x 427ns(cold) covers DMA-start latency (up to ~12us observed) so
    # the PE never idles >3.4us early on -- an early idle window would drop
    # the HAM clock gate back to 1.2 GHz and can poison the whole run.
    warm_ps = ps_w.tile([P, 512], F32, tag="ps_work", name="warm")
    for _ in range(20):
        nc.tensor.matmul(warm_ps[:1, :], ones16[:], warm_sb[:], start=True, stop=True)

    # Input DMA, ordered by first use; two HWDGE rings (sync/scalar).
    mt16_r = mt16.rearrange("(o p) n -> p o n", p=P)
    xt16k_r = xt16k.rearrange("(o p) n -> p o n", p=P)
    x8q_r = x8q.rearrange("(o p) n -> p o n", p=P)
    wv16_r = wv16.rearrange("(o p) n -> p o n", p=P)
    nc.sync.dma_start(mt16_sb[:, :, :128], mt16_r[:, :, :128])
    nc.scalar.dma_start(xt16k_sb[:, :, :512], xt16k_r[:, :, :512])
    nc.sync.dma_start(mt16_sb[:, :, 128:512], mt16_r[:, :, 128:512])
    nc.sync.dma_start(mt16_sb[:, :, 512:], mt16_r[:, :, 512:])
    nc.scalar.dma_start(xt16k_sb[:, :, 512:], xt16k_r[:, :, 512:])
    nc.scalar.dma_start(x8q_sb[:, :, :1024], x8q_r[:, :, :1024])
    nc.sync.dma_start(m16_sb[:], m16[:])
    nc.sync.dma_start(wv16_sb[:, :, :512], wv16_r[:, :, :512])
    nc.scalar.dma_start(x8q_sb[:, :, 1024:], x8q_r[:, :, 1024:])
    nc.sync.dma_start(wv16_sb[:, :, 512:], wv16_r[:, :, 512:])

    def proj16(lhs_sb, rhs_sb, m, n, name):
        """bf16 psum[m-block(128), n-block(512)] = lhsT.T @ rhs over dm."""
        ps = ps_w.tile([P, 512], F32, tag="ps_work", name=name)
        for kt in range(KT):
            nc.tensor.matmul(
                ps[:],
                lhs_sb[:, kt, m * P : (m + 1) * P],
                rhs_sb[:, kt, n * 512 : (n + 1) * 512],
                start=(kt == 0),
                stop=(kt == KT - 1),
            )
        return ps

    # --- h projection: h8[do, s] = 16 * (M @ x_s), this core's 1024 keys.
    for n in range(2):
        for m in range(KT):
            ps = proj16(mt16_sb, xt16k_sb, m, n, f"hp_{m}_{n}")
            nc.vector.tensor_scalar_mul(
                h8_sb[:, m, n * 512 : (n + 1) * 512], ps[:], SH
            )

    # --- attention row tile J (TJ=512 queries, orig rows 512J..512J+511):
    # key slots 0..2J+1; slots {2J, 2J+1} are diagonal (host-baked masks,
    # shift 256d + 128p).
    def scores_exp(J):
        es = []
        for s in range(2 * J + 2):
            sc = ps_w.tile([P, TJ], F32, tag="ps_work", name=f"sc_{J}_{s}")
            for kp in range(KT // 2):
                nc.tensor.matmul(
                    sc[:],
                    h8_sb[:, 2 * kp : 2 * kp + 2, s * P : (s + 1) * P],
                    x8q_sb[:, 2 * kp : 2 * kp + 2, J * TJ : (J + 1) * TJ],
                    start=(kp == 0),
                    stop=(kp == KT // 2 - 1),
                    perf_mode=DR,
                )
            e16 = epool.tile([P, TJ], BF16, tag="e16", name=f"e_{J}_{s}")
            nc.scalar.activation(
                e16[:], sc[:], mybir.ActivationFunctionType.Exp, scale=EXP_SCALE
            )
            d = s - 2 * J
            if d >= 0:  # diagonal slot
                nc.gpsimd.tensor_tensor(
                    e16[:], e16[:], m16_sb[:, d * TJ : (d + 1) * TJ],
                    mybir.AluOpType.mult,
                )
            es.append(e16)
        return es

    def sums_av_drain(J, es):
        nslots = 2 * J + 2
        sums_ps = ps_s.tile([1, TJ], F32, tag="ps_sums", name=f"su_{J}")
        for s in range(nslots):
            nc.tensor.matmul(
                sums_ps[:],
                ones16[:],
                es[s][:],
                start=(s == 0),
                stop=(s == nslots - 1),
            )
        nc.vector.tensor_copy(s_all[:, J, :], sums_ps[:])
        if J == NJ - 1:
            nc.sync.dma_start(sums[:], s_all[:, :, :])

        for dvh in range(2):
            av = [
                ps_av.tile([P, 512], F32, tag="ps_av", name=f"av_{J}_{c}_{dvh}")
                for c in range(4)
            ]
            for s in range(nslots):
                last_d1 = s == nslots - 1  # d=1 diagonal: c-blocks 0,1 all-zero
                for c in range(4):
                    if last_d1 and c < 2:
                        continue
                    nc.tensor.matmul(
                        av[c][:],
                        es[s][:, c * P : (c + 1) * P],
                        v16_sb[:, s, dvh * 512 : (dvh + 1) * 512],
                        start=(s == 0),
                        stop=(s == (nslots - 2 if c < 2 else nslots - 1)),
                    )
            for c in range(4):
                row = J * TJ + c * P
                o_sb = outp.tile([P, 512], BF16, tag="o_sb", name=f"o_{J}_{c}_{dvh}")
                if c % 2:
                    nc.scalar.activation(
                        o_sb[:], av[c][:], mybir.ActivationFunctionType.Copy
                    )
                else:
                    nc.vector.tensor_copy(o_sb[:], av[c][:])
                dma = nc.sync if (c + dvh) % 2 else nc.scalar
                dma.dma_start(
                    out_u[row : row + P, dvh * 512 : (dvh + 1) * 512], o_sb[:]
                )

    # Pipeline: scores(J) stay one step ahead of sums/AV(J-1) so exp
    # latency hides under PE work; v-projection slots in before the
    # first AV consumer.
    e0 = scores_exp(0)

    # --- v projection: v16[s, dv] (bf16).
    for m in range(NSLOT):
        for n in range(2):
            ps = proj16(xt16k_sb, wv16_sb, m, n, f"vp_{m}_{n}")
            nc.vector.tensor_copy(v16_sb[:, m, n * 512 : (n + 1) * 512], ps[:])

    e1 = scores_exp(1)
    sums_av_drain(0, e0)
    e2 = scores_exp(2)
    sums_av_drain(1, e1)
    e3 = scores_exp(3)
    sums_av_drain(2, e2)
    sums_av_drain(3, e3)


def _shard_inputs(x, wq, wk, wv):
    bf = ml_dtypes.bfloat16
    f8 = ml_dtypes.float8_e4m3
    M = wq.astype(np.float64) @ wk.astype(np.float64).T  # [do, di]
    mt16 = np.ascontiguousarray(M.T.astype(bf))  # [di, do]
    wv16 = np.ascontiguousarray(wv.astype(bf))
    l = np.arange(P)
    in_maps = []
    for b in range(B):
        for p in range(2):
            krows = np.concatenate(
                [np.arange(blk * P, blk * P + P) for blk in _key_blocks(p)]
            )
            xt = x[b].T  # [D, T] original order
            q = np.arange(TJ)
            m = np.concatenate(
                [
                    (q[None, :] >= l[:, None] + 256 * d + 128 * p).astype(np.float32)
                    for d in range(2)
                ],
                axis=1,
            )  # [128, 1024]
            in_maps.append(
                {
                    "x8q": np.ascontiguousarray((SX * xt).astype(f8)),
                    "xt16k": np.ascontiguousarray(xt[:, krows].astype(bf)),
                    "mt16": mt16,
                    "wv16": wv16,
                    "m16": np.ascontiguousarray(m.astype(bf)),
                }
            )
    return in_maps


def run(embedding_word, w_q, w_k, w_v, **spmd_kwargs):
    x = np.asarray(embedding_word, dtype=np.float32)
    assert x.shape == (B, T, D), x.shape
    if "nc" not in _NC_CACHE:
        _NC_CACHE["nc"] = _build_program()
    nc = _NC_CACHE["nc"]
    in_maps = _shard_inputs(
        x,
        np.asarray(w_q, np.float32),
        np.asarray(w_k, np.float32),
        np.asarray(w_v, np.float32),
    )
    # The accelerator occasionally reports a transient unrecoverable state
    # on the first touch from a fresh process; retry a couple of times.
    last_err = None
    for attempt in range(3):
        try:
            res = run_bass_kernel_spmd(
                nc, in_maps, core_ids=list(range(8)), **spmd_kwargs
            )
            break
        except Exception as err:  # pragma: no cover
            last_err = err
            import time

            time.sleep(5.0 * (attempt + 1))
    else:
        raise last_err

    out = np.empty((B, T, D), np.float32)
    for b in range(B):
        u = res.results[2 * b]["out_u"].astype(np.float32) + res.results[
            2 * b + 1
        ]["out_u"].astype(np.float32)
        s = (
            res.results[2 * b]["sums"].reshape(T)
            + res.results[2 * b + 1]["sums"].reshape(T)
        )
        out[b] = u / s[:, None]
    return out, res


def kernel(embedding_word, w_q, w_k, w_v):
    out, _ = run(embedding_word, w_q, w_k, w_v)
    return out


# revision 17
# speedup vs baseline: 1.1684x; 1.0003x over previous
"""Causal single-head attention on 8 Trainium2 NeuronCores.

Problem: embedding_word [4, 2048, 1024] fp32; w_q/w_k/w_v [1024, 1024] fp32.
  q = x @ w_q; k = x @ w_k; v = x @ w_v
  out = softmax(causal_mask(q k^T) / 32) @ v          per batch.

Sharding: 4 batches x 2 key-shards = 8 cores (SPMD, one program).
Core (b, p) handles batch b and the interleaved key blocks
{128*(2i+p) : i in 0..7} (1024 keys), for ALL 2048 query rows, producing
the *unnormalized* attention output u = sum_s exp(score) * v[s] and the
per-row sum of exp.  Host combines the two key-shards per batch:
  out = (u_p0 + u_p1) / (s_p0 + s_p1).
Scores are bounded (|score/32| < ~2), so softmax without max-subtraction
is numerically safe and the partial sums combine linearly.

Two tricks vs a plain bf16 implementation:

1. M-folding: scores = q k^T = x (w_q w_k^T) x^T.  The host precomputes
   M = w_q w_k^T in fp64; the kernel computes h = x_keys @ M^T (one
   k-projection-sized matmul) and scores = x @ h^T.  The entire
   q-projection (a third of all projection FLOPs, and the only
   projection needing all 2048 tokens) disappears.

2. fp8-e4m3 DoubleRow scores: the score matmuls (a quarter of the
   attention FLOPs) run with 2 contraction elements per PE cell-cycle
   (K=256 per instruction) on x8 = fp8(16x) and h8 = fp8(16h).  Score
   error ~2.5% of logit scale shifts softmax weights by ~2.5% relative,
   which the consistent e/sums normalization mostly cancels; measured
   end-to-end error is ~1e-2 absmax-relative / ~1.2e-2 Frobenius.
   Everything touching v stays bf16 (v-quantization error does NOT
   average out of a relative-error metric).

Layout: the score moving operand x8 is in ORIGINAL token order, so row
tiles (TJ=512 queries) are contiguous slices, output rows land in
original order (no host unpermute), and causal masks are simple shifted
triangles baked per-core by the host ([128, 2*512]: diagonal slot
2J+d is masked with m16[:, 512d : 512d+512], = (q >= r + 256d + 128p)).
The e/sums/AV path is uniform bf16 for all row tiles; AV matmuls that
are fully masked out ((d=1 diagonal slot) x (query c-blocks 0,1)) are
skipped on both parities.
"""

import numpy as np
import ml_dtypes

try:
    import concourse.bass as bass  # noqa: F401
except ImportError:  # pragma: no cover
    import sys

    sys.path.insert(0, "/opt/trn_rl_repo")
    import concourse.bass as bass  # noqa: F401

from contextlib import ExitStack

import concourse.tile as tile
from concourse import bacc, mybir
from concourse.bass_utils import run_bass_kernel_spmd

B = 4
T = 2048
D = 1024
P = 128
KT = D // P  # 8 contraction subtiles of 128
NSLOT = 8  # key slots per core (each 128 keys)
TJ = 512  # query rows per attention tile
NJ = T // TJ  # 4 row tiles
BF16 = mybir.dt.bfloat16
F8 = mybir.dt.float8e4
F32 = mybir.dt.float32
DR = mybir.MatmulPerfMode.DoubleRow

SX = 16.0  # x8 = SX * x^T
SH = 16.0  # h8 = SH * h
EXP_SCALE = (1.0 / 32.0) / (SX * SH)  # score psum -> logits

_NC_CACHE = {}


def _key_blocks(p):
    """Key slot i (0..7) -> original 128-row block index."""
    return [2 * i + p for i in range(NSLOT)]


def _build_program():
    nc = bacc.Bacc(
        "TRN2",
        target_bir_lowering=False,
        debug=False,
        enable_asserts=False,
        num_devices=8,
    )
    x8q = nc.dram_tensor("x8q", [D, T], F8, kind="ExternalInput").ap()
    xt16k = nc.dram_tensor("xt16k", [D, NSLOT * P], BF16, kind="ExternalInput").ap()
    mt16 = nc.dram_tensor("mt16", [D, D], BF16, kind="ExternalInput").ap()
    wv16 = nc.dram_tensor("wv16", [D, D], BF16, kind="ExternalInput").ap()
    m16 = nc.dram_tensor("m16", [P, 2 * TJ], BF16, kind="ExternalInput").ap()
    out_u = nc.dram_tensor("out_u", [T, D], BF16, kind="ExternalOutput").ap()
    sums = nc.dram_tensor("sums", [1, T], F32, kind="ExternalOutput").ap()

    with tile.TileContext(nc) as tc, ExitStack() as ctx:
        _emit(ctx, tc, x8q, xt16k, mt16, wv16, m16, out_u, sums)
    nc.compile()
    return nc


def _emit(ctx, tc, x8q, xt16k, mt16, wv16, m16, out_u, sums):
    nc = tc.nc

    const = ctx.enter_context(tc.tile_pool(name="const", bufs=1))
    big = ctx.enter_context(tc.tile_pool(name="big", bufs=1))
    epool = ctx.enter_context(tc.tile_pool(name="epool", bufs=15))
    outp = ctx.enter_context(tc.tile_pool(name="outp", bufs=6))
    ps_w = ctx.enter_context(tc.tile_pool(name="ps_w", bufs=2, space="PSUM"))
    ps_av = ctx.enter_context(tc.tile_pool(name="ps_av", bufs=5, space="PSUM"))
    ps_s = ctx.enter_context(tc.tile_pool(name="ps_s", bufs=1, space="PSUM"))

    # Persistent SBUF tensors (layout [128 partitions, outer, free]).
    x8q_sb = big.tile([P, KT, T], F8)  # 16*x^T [dm_p, dm_o, t(ORIG)]
    xt16k_sb = big.tile([P, KT, NSLOT * P], BF16)  # x^T keys [dm_p, dm_o, s]
    mt16_sb = big.tile([P, KT, D], BF16)  # M^T    [di_p, di_o, do]
    wv16_sb = big.tile([P, KT, D], BF16)
    h8_sb = big.tile([P, KT, NSLOT * P], F8)  # 16*h^T [do_p, do_o, s]
    v16_sb = big.tile([P, NSLOT, D], BF16)  # v      [s_p, s_o, dv]
    m16_sb = const.tile([P, 2 * TJ], BF16)
    ones16 = const.tile([P, 1], BF16)
    s_all = const.tile([1, NJ, TJ], F32)

    nc.vector.memset(ones16[:], 1.0)
    # Warm-up: keep the PE busy while the first input DMAs land so the
    # HAM clock gate reaches 2.4 GHz before real work starts.
    warm_sb = const.tile([P, 512], BF16)
    nc.vector.memset(warm_sb[:], 0.0)
    # ~20 x 427ns(cold) covers typical DMA-start latency (~5-9us) so the
    # PE never idles >3.4us early on -- an early idle window would drop
    # the HAM clock gate back to 1.2 GHz and can poison the whole run.
    warm_ps = ps_w.tile([P, 512], F32, tag="ps_work", name="warm")
    for _ in range(20):
        nc.tensor.matmul(warm_ps[:1, :], ones16[:], warm_sb[:], start=True, stop=True)

    # Input DMA, ordered by first use; two HWDGE rings (sync/scalar).
    mt16_r = mt16.rearrange("(o p) n -> p o n", p=P)
    xt16k_r = xt16k.rearrange("(o p) n -> p o n", p=P)
    x8q_r = x8q.rearrange("(o p) n -> p o n", p=P)
    wv16_r = wv16.rearrange("(o p) n -> p o n", p=P)
    nc.sync.dma_start(mt16_sb[:, :, :128], mt16_r[:, :, :128])
    nc.scalar.dma_start(xt16k_sb[:, :, :512], xt16k_r[:, :, :512])
    nc.sync.dma_start(mt16_sb[:, :, 128:512], mt16_r[:, :, 128:512])
    nc.sync.dma_start(mt16_sb[:, :, 512:], mt16_r[:, :, 512:])
    nc.scalar.dma_start(xt16k_sb[:, :, 512:], xt16k_r[:, :, 512:])
    nc.scalar.dma_start(x8q_sb[:, :, :1024], x8q_r[:, :, :1024])
    nc.sync.dma_start(m16_sb[:], m16[:])
    nc.sync.dma_start(wv16_sb[:, :, :512], wv16_r[:, :, :512])
    nc.scalar.dma_start(x8q_sb[:, :, 1024:], x8q_r[:, :, 1024:])
    nc.sync.dma_start(wv16_sb[:, :, 512:], wv16_r[:, :, 512:])

    def proj16(lhs_sb, rhs_sb, m, n, name):
        """bf16 psum[m-block(128), n-block(512)] = lhsT.T @ rhs over dm."""
        ps = ps_w.tile([P, 512], F32, tag="ps_work", name=name)
        for kt in range(KT):
            nc.tensor.matmul(
                ps[:],
                lhs_sb[:, kt, m * P : (m + 1) * P],
                rhs_sb[:, kt, n * 512 : (n + 1) * 512],
                start=(kt == 0),
                stop=(kt == KT - 1),
            )
        return ps

    # --- h projection: h8[do, s] = 16 * (M @ x_s), this core's 1024 keys.
    for n in range(2):
        for m in range(KT):
            ps = proj16(mt16_sb, xt16k_sb, m, n, f"hp_{m}_{n}")
            nc.vector.tensor_scalar_mul(
                h8_sb[:, m, n * 512 : (n + 1) * 512], ps[:], SH
            )

    # --- attention row tile J (TJ=512 queries, orig rows 512J..512J+511):
    # key slots 0..2J+1; slots {2J, 2J+1} are diagonal (host-baked masks,
    # shift 256d + 128p).
    def scores_exp(J):
        es = []
        for s in range(2 * J + 2):
            sc = ps_w.tile([P, TJ], F32, tag="ps_work", name=f"sc_{J}_{s}")
            for kp in range(KT // 2):
                nc.tensor.matmul(
                    sc[:],
                    h8_sb[:, 2 * kp : 2 * kp + 2, s * P : (s + 1) * P],
                    x8q_sb[:, 2 * kp : 2 * kp + 2, J * TJ : (J + 1) * TJ],
                    start=(kp == 0),
                    stop=(kp == KT // 2 - 1),
                    perf_mode=DR,
                )
            e16 = epool.tile([P, TJ], BF16, tag="e16", name=f"e_{J}_{s}")
            nc.scalar.activation(
                e16[:], sc[:], mybir.ActivationFunctionType.Exp, scale=EXP_SCALE
            )
            d = s - 2 * J
            if d >= 0:  # diagonal slot
                nc.gpsimd.tensor_tensor(
                    e16[:], e16[:], m16_sb[:, d * TJ : (d + 1) * TJ],
                    mybir.AluOpType.mult,
                )
            es.append(e16)
        return es

    def sums_av_drain(J, es):
        nslots = 2 * J + 2
        sums_ps = ps_s.tile([1, TJ], F32, tag="ps_sums", name=f"su_{J}")
        for s in range(nslots):
            nc.tensor.matmul(
                sums_ps[:],
                ones16[:],
                es[s][:],
                start=(s == 0),
                stop=(s == nslots - 1),
            )
        nc.vector.tensor_copy(s_all[:, J, :], sums_ps[:])
        if J == NJ - 1:
            nc.sync.dma_start(sums[:], s_all[:, :, :])

        for dvh in range(2):
            av = [
                ps_av.tile([P, 512], F32, tag="ps_av", name=f"av_{J}_{c}_{dvh}")
                for c in range(4)
            ]
            for s in range(nslots):
                last_d1 = s == nslots - 1  # d=1 diagonal: c-blocks 0,1 all-zero
                for c in range(4):
                    if last_d1 and c < 2:
                        continue
                    nc.tensor.matmul(
                        av[c][:],
                        es[s][:, c * P : (c + 1) * P],
                        v16_sb[:, s, dvh * 512 : (dvh + 1) * 512],
                        start=(s == 0),
                        stop=(s == (nslots - 2 if c < 2 else nslots - 1)),
                    )
            for c in range(4):
                row = J * TJ + c * P
                o_sb = outp.tile([P, 512], BF16, tag="o_sb", name=f"o_{J}_{c}_{dvh}")
                if c % 2:
                    nc.scalar.activation(
                        o_sb[:], av[c][:], mybir.ActivationFunctionType.Copy
                    )
                else:
                    nc.vector.tensor_copy(o_sb[:], av[c][:])
                dma = nc.sync if (c + dvh) % 2 else nc.scalar
                dma.dma_start(
                    out_u[row : row + P, dvh * 512 : (dvh + 1) * 512], o_sb[:]
                )

    # Pipeline: scores(J) stay one step ahead of sums/AV(J-1) so exp
    # latency hides under PE work; v-projection slots in before the
    # first AV consumer.
    e0 = scores_exp(0)

    # --- v projection: v16[s, dv] (bf16).
    for m in range(NSLOT):
        for n in range(2):
            ps = proj16(xt16k_sb, wv16_sb, m, n, f"vp_{m}_{n}")
            nc.vector.tensor_copy(v16_sb[:, m, n * 512 : (n + 1) * 512], ps[:])

    e1 = scores_exp(1)
    sums_av_drain(0, e0)
    e2 = scores_exp(2)
    sums_av_drain(1, e1)
    e3 = scores_exp(3)
    sums_av_drain(2, e2)
    sums_av_drain(3, e3)


def _shard_inputs(x, wq, wk, wv):
    bf = ml_dtypes.bfloat16
    f8 = ml_dtypes.float8_e4m3
    M = wq.astype(np.float64) @ wk.astype(np.float64).T  # [do, di]
    mt16 = np.ascontiguousarray(M.T.astype(bf))  # [di, do]
    wv16 = np.ascontiguousarray(wv.astype(bf))
    l = np.arange(P)
    in_maps = []
    for b in range(B):
        for p in range(2):
            krows = np.concatenate(
                [np.arange(blk * P, blk * P + P) for blk in _key_blocks(p)]
            )
            xt = x[b].T  # [D, T] original order
            q = np.arange(TJ)
            m = np.concatenate(
                [
                    (q[None, :] >= l[:, None] + 256 * d + 128 * p).astype(np.float32)
                    for d in range(2)
                ],
                axis=1,
            )  # [128, 1024]
            in_maps.append(
                {
                    "x8q": np.ascontiguousarray((SX * xt).astype(f8)),
                    "xt16k": np.ascontiguousarray(xt[:, krows].astype(bf)),
                    "mt16": mt16,
                    "wv16": wv16,
                    "m16": np.ascontiguousarray(m.astype(bf)),
                }
            )
    return in_maps


def run(embedding_word, w_q, w_k, w_v, **spmd_kwargs):
    x = np.asarray(embedding_word, dtype=np.float32)
    assert x.shape == (B, T, D), x.shape
    if "nc" not in _NC_CACHE:
        _NC_CACHE["nc"] = _build_program()
    nc = _NC_CACHE["nc"]
    in_maps = _shard_inputs(
        x,
        np.asarray(w_q, np.float32),
        np.asarray(w_k, np.float32),
        np.asarray(w_v, np.float32),
    )
    # The accelerator occasionally reports a transient unrecoverable state
    # on the first touch from a fresh process; retry a couple of times.
    last_err = None
    for attempt in range(3):
        try:
            res = run_bass_kernel_spmd(
                nc, in_maps, core_ids=list(range(8)), **spmd_kwargs
            )
            break
        except Exception as err:  # pragma: no cover
            last_err = err
            import time

            time.sleep(5.0 * (attempt + 1))
    else:
        raise last_err

    out = np.empty((B, T, D), np.float32)
    for b in range(B):
        u = res.results[2 * b]["out_u"].astype(np.float32) + res.results[
            2 * b + 1
        ]["out_u"].astype(np.float32)
        s = (
            res.results[2 * b]["sums"].reshape(T)
            + res.results[2 * b + 1]["sums"].reshape(T)
        )
        out[b] = u / s[:, None]
    return out, res


def kernel(embedding_word, w_q, w_k, w_v):
    out, _ = run(embedding_word, w_q, w_k, w_v)
    return out


# revision 25
# speedup vs baseline: 1.1726x; 1.0036x over previous
"""Causal single-head attention on 8 Trainium2 NeuronCores.

Problem: embedding_word [4, 2048, 1024] fp32; w_q/w_k/w_v [1024, 1024] fp32.
  q = x @ w_q; k = x @ w_k; v = x @ w_v
  out = softmax(causal_mask(q k^T) / 32) @ v          per batch.

Sharding: 4 batches x 2 key-shards = 8 cores (SPMD, one program).
Core (b, p) handles batch b and the interleaved key blocks
{128*(2i+p) : i in 0..7} (1024 keys), for ALL 2048 query rows, producing
the *unnormalized* attention output u = sum_s exp(score) * v[s] and the
per-row sum of exp.  Host combines the two key-shards per batch:
  out = (u_p0 + u_p1) / (s_p0 + s_p1).
Scores are bounded (|score/32| < ~2), so softmax without max-subtraction
is numerically safe and the partial sums combine linearly.

Two tricks vs a plain bf16 implementation:

1. M-folding: scores = q k^T = x (w_q w_k^T) x^T.  The host precomputes
   M = w_q w_k^T in fp64; the kernel computes h = x_keys @ M^T (one
   k-projection-sized matmul) and scores = x @ h^T.  The entire
   q-projection (a third of all projection FLOPs, and the only
   projection needing all 2048 tokens) disappears.

2. fp8-e4m3 DoubleRow scores: the score matmuls (a quarter of the
   attention FLOPs) run with 2 contraction elements per PE cell-cycle
   (K=256 per instruction) on x8 = fp8(16x) and h8 = fp8(16h).  Score
   error ~2.5% of logit scale shifts softmax weights by ~2.5% relative,
   which the consistent e/sums normalization mostly cancels; measured
   end-to-end error is ~1e-2 absmax-relative / ~1.2e-2 Frobenius.
   Everything touching v stays bf16 (v-quantization error does NOT
   average out of a relative-error metric).

Layout: the score moving operand x8 is in ORIGINAL token order, so row
tiles (TJ=512 queries) are contiguous slices, output rows land in
original order (no host unpermute), and causal masks are simple shifted
triangles baked per-core by the host ([128, 2*512]: diagonal slot
2J+d is masked with m16[:, 512d : 512d+512], = (q >= r + 256d + 128p)).
The e/sums/AV path is uniform bf16 for all row tiles; AV matmuls that
are fully masked out ((d=1 diagonal slot) x (query c-blocks 0,1)) are
skipped on both parities.
"""

import numpy as np
import ml_dtypes

try:
    import concourse.bass as bass  # noqa: F401
except ImportError:  # pragma: no cover
    import sys

    sys.path.insert(0, "/opt/trn_rl_repo")
    import concourse.bass as bass  # noqa: F401

from contextlib import ExitStack

import concourse.tile as tile
from concourse import bacc, mybir
from concourse.bass_utils import run_bass_kernel_spmd

B = 4
T = 2048
D = 1024
P = 128
KT = D // P  # 8 contraction subtiles of 128
NSLOT = 8  # key slots per core (each 128 keys)
TJ = 512  # query rows per attention tile
NJ = T // TJ  # 4 row tiles
BF16 = mybir.dt.bfloat16
F8 = mybir.dt.float8e4
F32 = mybir.dt.float32
DR = mybir.MatmulPerfMode.DoubleRow

SX = 16.0  # x8 = SX * x^T
SH = 16.0  # h8 = SH * h
EXP_SCALE = (1.0 / 32.0) / (SX * SH)  # score psum -> logits

_NC_CACHE = {}


def _key_blocks(p):
    """Key slot i (0..7) -> original 128-row block index."""
    return [2 * i + p for i in range(NSLOT)]


def _build_program():
    nc = bacc.Bacc(
        "TRN2",
        target_bir_lowering=False,
        debug=False,
        enable_asserts=False,
        num_devices=8,
    )
    x8q = nc.dram_tensor("x8q", [D, T], F8, kind="ExternalInput").ap()
    xt16k = nc.dram_tensor("xt16k", [D, NSLOT * P], BF16, kind="ExternalInput").ap()
    mt16 = nc.dram_tensor("mt16", [D, D], BF16, kind="ExternalInput").ap()
    wv16 = nc.dram_tensor("wv16", [D, D], BF16, kind="ExternalInput").ap()
    m16 = nc.dram_tensor("m16", [P, 2 * TJ], BF16, kind="ExternalInput").ap()
    out_u = nc.dram_tensor("out_u", [T, D], BF16, kind="ExternalOutput").ap()
    sums = nc.dram_tensor("sums", [1, T], F32, kind="ExternalOutput").ap()

    with tile.TileContext(nc) as tc, ExitStack() as ctx:
        _emit(ctx, tc, x8q, xt16k, mt16, wv16, m16, out_u, sums)
    nc.compile()
    return nc


def _emit(ctx, tc, x8q, xt16k, mt16, wv16, m16, out_u, sums):
    nc = tc.nc

    const = ctx.enter_context(tc.tile_pool(name="const", bufs=1))
    big = ctx.enter_context(tc.tile_pool(name="big", bufs=1))
    epool = ctx.enter_context(tc.tile_pool(name="epool", bufs=15))
    outp = ctx.enter_context(tc.tile_pool(name="outp", bufs=6))
    ps_w = ctx.enter_context(tc.tile_pool(name="ps_w", bufs=3, space="PSUM"))
    ps_av = ctx.enter_context(tc.tile_pool(name="ps_av", bufs=4, space="PSUM"))
    ps_s = ctx.enter_context(tc.tile_pool(name="ps_s", bufs=1, space="PSUM"))

    # Persistent SBUF tensors (layout [128 partitions, outer, free]).
    x8q_sb = big.tile([P, KT, T], F8)  # 16*x^T [dm_p, dm_o, t(ORIG)]
    xt16k_sb = big.tile([P, KT, NSLOT * P], BF16)  # x^T keys [dm_p, dm_o, s]
    mt16_sb = big.tile([P, KT, D], BF16)  # M^T    [di_p, di_o, do]
    wv16_sb = big.tile([P, KT, D], BF16)
    h8_sb = big.tile([P, KT, NSLOT * P], F8)  # 16*h^T [do_p, do_o, s]
    v16_sb = big.tile([P, NSLOT, D], BF16)  # v      [s_p, s_o, dv]
    m16_sb = const.tile([P, 2 * TJ], BF16)
    ones16 = const.tile([P, 1], BF16)
    s_all = const.tile([1, NJ, TJ], F32)

    nc.vector.memset(ones16[:], 1.0)
    # Warm-up: keep the PE busy while the first input DMAs land so the
    # HAM clock gate reaches 2.4 GHz before real work starts.
    warm_sb = const.tile([P, 512], BF16)
    nc.vector.memset(warm_sb[:], 0.0)
    # ~14 x 427ns(cold) covers typical DMA-start latency so the PE never
    # idles >3.4us early on -- an early idle window would drop the HAM
    # clock gate back to 1.2 GHz and can poison the whole run.
    warm_ps = ps_w.tile([P, 512], F32, tag="ps_work", name="warm")
    for _ in range(14):
        nc.tensor.matmul(warm_ps[:1, :], ones16[:], warm_sb[:], start=True, stop=True)

    # Input DMA, ordered by first use; two HWDGE rings (sync/scalar).
    mt16_r = mt16.rearrange("(o p) n -> p o n", p=P)
    xt16k_r = xt16k.rearrange("(o p) n -> p o n", p=P)
    x8q_r = x8q.rearrange("(o p) n -> p o n", p=P)
    wv16_r = wv16.rearrange("(o p) n -> p o n", p=P)
    nc.sync.dma_start(mt16_sb[:, :, :256], mt16_r[:, :, :256])
    nc.scalar.dma_start(xt16k_sb[:, :, :512], xt16k_r[:, :, :512])
    nc.sync.dma_start(mt16_sb[:, :, 256:], mt16_r[:, :, 256:])
    nc.scalar.dma_start(xt16k_sb[:, :, 512:], xt16k_r[:, :, 512:])
    nc.scalar.dma_start(x8q_sb[:, :, :1024], x8q_r[:, :, :1024])
    nc.sync.dma_start(m16_sb[:], m16[:])
    nc.sync.dma_start(wv16_sb[:, :, :512], wv16_r[:, :, :512])
    nc.scalar.dma_start(x8q_sb[:, :, 1024:], x8q_r[:, :, 1024:])
    nc.sync.dma_start(wv16_sb[:, :, 512:], wv16_r[:, :, 512:])

    def proj16(lhs_sb, rhs_sb, m, n, name):
        """bf16 psum[m-block(128), n-block(512)] = lhsT.T @ rhs over dm."""
        ps = ps_w.tile([P, 512], F32, tag="ps_work", name=name)
        for kt in range(KT):
            nc.tensor.matmul(
                ps[:],
                lhs_sb[:, kt, m * P : (m + 1) * P],
                rhs_sb[:, kt, n * 512 : (n + 1) * 512],
                start=(kt == 0),
                stop=(kt == KT - 1),
            )
        return ps

    # --- h projection: h8[do, s] = 16 * (M @ x_s), this core's 1024 keys.
    for n in range(2):
        for m in range(KT):
            ps = proj16(mt16_sb, xt16k_sb, m, n, f"hp_{m}_{n}")
            nc.vector.tensor_scalar_mul(
                h8_sb[:, m, n * 512 : (n + 1) * 512], ps[:], SH
            )

    # --- attention row tile J (TJ=512 queries, orig rows 512J..512J+511):
    # key slots 0..2J+1; slots {2J, 2J+1} are diagonal (host-baked masks,
    # shift 256d + 128p).
    def scores_exp(J):
        es = []
        for s in range(2 * J + 2):
            sc = ps_w.tile([P, TJ], F32, tag="ps_work", name=f"sc_{J}_{s}")
            for kp in range(KT // 2):
                nc.tensor.matmul(
                    sc[:],
                    h8_sb[:, 2 * kp : 2 * kp + 2, s * P : (s + 1) * P],
                    x8q_sb[:, 2 * kp : 2 * kp + 2, J * TJ : (J + 1) * TJ],
                    start=(kp == 0),
                    stop=(kp == KT // 2 - 1),
                    perf_mode=DR,
                )
            e16 = epool.tile([P, TJ], BF16, tag="e16", name=f"e_{J}_{s}")
            nc.scalar.activation(
                e16[:], sc[:], mybir.ActivationFunctionType.Exp, scale=EXP_SCALE
            )
            d = s - 2 * J
            if d >= 0:  # diagonal slot
                nc.gpsimd.tensor_tensor(
                    e16[:], e16[:], m16_sb[:, d * TJ : (d + 1) * TJ],
                    mybir.AluOpType.mult,
                )
            es.append(e16)
        return es

    def sums_av_drain(J, es):
        nslots = 2 * J + 2
        sums_ps = ps_s.tile([1, TJ], F32, tag="ps_sums", name=f"su_{J}")
        for s in range(nslots):
            nc.tensor.matmul(
                sums_ps[:],
                ones16[:],
                es[s][:],
                start=(s == 0),
                stop=(s == nslots - 1),
            )
        nc.vector.tensor_copy(s_all[:, J, :], sums_ps[:])
        if J == NJ - 1:
            nc.sync.dma_start(sums[:], s_all[:, :, :])

        for dvh in range(2):
            av = [
                ps_av.tile([P, 512], F32, tag="ps_av", name=f"av_{J}_{c}_{dvh}")
                for c in range(4)
            ]
            for s in range(nslots):
                last_d1 = s == nslots - 1  # d=1 diagonal: c-blocks 0,1 all-zero
                for c in range(4):
                    if last_d1 and c < 2:
                        continue
                    nc.tensor.matmul(
                        av[c][:],
                        es[s][:, c * P : (c + 1) * P],
                        v16_sb[:, s, dvh * 512 : (dvh + 1) * 512],
                        start=(s == 0),
                        stop=(s == (nslots - 2 if c < 2 else nslots - 1)),
                    )
            for c in range(4):
                row = J * TJ + c * P
                o_sb = outp.tile([P, 512], BF16, tag="o_sb", name=f"o_{J}_{c}_{dvh}")
                if c % 2:
                    nc.scalar.activation(
                        o_sb[:], av[c][:], mybir.ActivationFunctionType.Copy
                    )
                else:
                    nc.vector.tensor_copy(o_sb[:], av[c][:])
                dma = nc.sync if (c + dvh) % 2 else nc.scalar
                dma.dma_start(
                    out_u[row : row + P, dvh * 512 : (dvh + 1) * 512], o_sb[:]
                )

    # Pipeline: scores(J) stay one step ahead of sums/AV(J-1) so exp
    # latency hides under PE work; v-projection slots in before the
    # first AV consumer.
    e0 = scores_exp(0)

    # --- v projection: v16[s, dv] (bf16).
    for m in range(NSLOT):
        for n in range(2):
            ps = proj16(xt16k_sb, wv16_sb, m, n, f"vp_{m}_{n}")
            nc.vector.tensor_copy(v16_sb[:, m, n * 512 : (n + 1) * 512], ps[:])

    e1 = scores_exp(1)
    sums_av_drain(0, e0)
    e2 = scores_exp(2)
    sums_av_drain(1, e1)
    e3 = scores_exp(3)
    sums_av_drain(2, e2)
    sums_av_drain(3, e3)


def _shard_inputs(x, wq, wk, wv):
    bf = ml_dtypes.bfloat16
    f8 = ml_dtypes.float8_e4m3
    M = wq.astype(np.float64) @ wk.astype(np.float64).T  # [do, di]
    mt16 = np.ascontiguousarray(M.T.astype(bf))  # [di, do]
    wv16 = np.ascontiguousarray(wv.astype(bf))
    l = np.arange(P)
    in_maps = []
    for b in range(B):
        for p in range(2):
            krows = np.concatenate(
                [np.arange(blk * P, blk * P + P) for blk in _key_blocks(p)]
            )
            xt = x[b].T  # [D, T] original order
            q = np.arange(TJ)
            m = np.concatenate(
                [
                    (q[None, :] >= l[:, None] + 256 * d + 128 * p).astype(np.float32)
                    for d in range(2)
                ],
                axis=1,
            )  # [128, 1024]
            in_maps.append(
                {
                    "x8q": np.ascontiguousarray((SX * xt).astype(f8)),
                    "xt16k": np.ascontiguousarray(xt[:, krows].astype(bf)),
                    "mt16": mt16,
                    "wv16": wv16,
                    "m16": np.ascontiguousarray(m.astype(bf)),
                }
            )
    return in_maps


def run(embedding_word, w_q, w_k, w_v, **spmd_kwargs):
    x = np.asarray(embedding_word, dtype=np.float32)
    assert x.shape == (B, T, D), x.shape
    if "nc" not in _NC_CACHE:
        _NC_CACHE["nc"] = _build_program()
    nc = _NC_CACHE["nc"]
    in_maps = _shard_inputs(
        x,
        np.asarray(w_q, np.float32),
        np.asarray(w_k, np.float32),
        np.asarray(w_v, np.float32),
    )
    # The accelerator occasionally reports a transient unrecoverable state
    # on the first touch from a fresh process; retry a couple of times.
    last_err = None
    for attempt in range(3):
        try:
            res = run_bass_kernel_spmd(
                nc, in_maps, core_ids=list(range(8)), **spmd_kwargs
            )
            break
        except Exception as err:  # pragma: no cover
            last_err = err
            import time

            time.sleep(5.0 * (attempt + 1))
    else:
        raise last_err

    out = np.empty((B, T, D), np.float32)
    for b in range(B):
        u = res.results[2 * b]["out_u"].astype(np.float32) + res.results[
            2 * b + 1
        ]["out_u"].astype(np.float32)
        s = (
            res.results[2 * b]["sums"].reshape(T)
            + res.results[2 * b + 1]["sums"].reshape(T)
        )
        out[b] = u / s[:, None]
    return out, res


def kernel(embedding_word, w_q, w_k, w_v):
    out, _ = run(embedding_word, w_q, w_k, w_v)
    return out
